# revision 1
# baseline (speedup 1.0000x reference)
"""Multi-head attention (B=2, L=4096, D=512, H=8, HD=64) on 8 trn2 NeuronCores.

Sharding: data-parallel over batch (2) x tensor-parallel over head-pairs (4):
core c handles batch c//4, heads (c%4)*2 and (c%4)*2+1. Each core projects
Q/K/V for its two heads (columns of Wq/Wk/Wv), runs flash-style attention
(S^T orientation, no-max-subtraction exp since logits are small, denominator
via an appended ones-column in V), applies its rows of Wo, and returns a
partial [L, D] output. Host sums the 4 partials per batch and adds bo.

Precision: S-path matmuls (projections + scores) use bf16 operands (weight
loads hide via FWL + the PE reorder window; score errors are softmax-damped);
the P@V and Wo matmuls use float32r (rounded fp32, full PE speed at N>=512,
~1e-4 matmul precision). PSUM accumulation is fp32 throughout.
"""

import sys
import types

import numpy as np

B, L, D = 2, 4096, 512
H, HD = 8, 64
NCORES = 8
HPC = 2          # heads per core
HD2 = HPC * HD   # 128
QB = 512         # query block (free dim of S^T tiles per head)
NQB = L // QB    # 8
KC = 128         # key-position chunk (partition dim of S^T tiles)
NKC = L // KC    # 32
NDC = D // 128   # 4 contraction chunks for projections

_CACHED_NC = None


def _ensure_axon_hook():
    """Register the NTFF profile hook boot() couldn't (stub antenv lacks
    axon_hooks). Harmless when tracing is never requested."""
    try:
        from antenv.axon_hooks import get_axon_ntff_profile_hook  # noqa: F401
        return
    except ImportError:
        pass
    hook = None
    try:
        from trn_agent_boot.trn_boot import _ntff_profile_via_ctypes
        hook = _ntff_profile_via_ctypes("/opt/axon/libaxon_pjrt.so")
    except Exception:
        pass
    mod = types.ModuleType("antenv.axon_hooks")
    mod.get_axon_ntff_profile_hook = lambda: hook
    mod.set_axon_ntff_profile_hook = lambda h: None
    sys.modules["antenv.axon_hooks"] = mod


def _build_nc():
    from concourse import bacc
    import concourse.mybir as mybir
    import concourse.tile as tile

    f32 = mybir.dt.float32
    f32r = mybir.dt.float32r
    bf16 = mybir.dt.bfloat16
    AF = mybir.ActivationFunctionType

    nc = bacc.Bacc("TRN2", target_bir_lowering=False, debug=False,
                   num_devices=NCORES)

    xq = nc.dram_tensor("xq", [D, L], f32, kind="ExternalInput")
    xk = nc.dram_tensor("xk", [D, L], f32, kind="ExternalInput")
    xv = nc.dram_tensor("xv", [D, L], f32, kind="ExternalInput")
    wq = nc.dram_tensor("wq", [D, HD2], f32, kind="ExternalInput")
    wk = nc.dram_tensor("wk", [D, HD2], f32, kind="ExternalInput")
    wv = nc.dram_tensor("wv", [D, HD2], f32, kind="ExternalInput")
    wo0 = nc.dram_tensor("wo0", [HD, D], f32, kind="ExternalInput")
    wo1 = nc.dram_tensor("wo1", [HD, D], f32, kind="ExternalInput")
    bq = nc.dram_tensor("bq", [HD2, 1], f32, kind="ExternalInput")
    bk = nc.dram_tensor("bk", [HD2, 1], f32, kind="ExternalInput")
    bvb = nc.dram_tensor("bvb", [128, HD2], f32, kind="ExternalInput")
    mb = nc.dram_tensor("mb", [KC, NKC], f32, kind="ExternalInput")
    out = nc.dram_tensor("out", [L, D], f32, kind="ExternalOutput")

    with tile.TileContext(nc) as tc:
        with (
            tc.tile_pool(name="singles", bufs=1) as singles,
            tc.tile_pool(name="xload", bufs=10) as xload,
            tc.tile_pool(name="xcast", bufs=10) as xcast,
            tc.tile_pool(name="qtp", bufs=NQB) as qtp,
            tc.tile_pool(name="ptp", bufs=4) as ptp,
            tc.tile_pool(name="xtp", bufs=4) as xtp,
            tc.tile_pool(name="op", bufs=3) as op,
            tc.tile_pool(name="small", bufs=4) as small,
            tc.tile_pool(name="dscr", bufs=2, space="DRAM") as dscr,
            tc.tile_pool(name="ps_s", bufs=2, space="PSUM") as ps_sp,
            tc.tile_pool(name="ps_u", bufs=4, space="PSUM") as ps_up,
        ):
            # ---------------- constants / weights ----------------
            def load_w(name, dram):
                wf = singles.tile([128, NDC, HD2], f32, tag=name + "f")
                nc.sync.dma_start(wf[:], dram.rearrange("(c p) m -> p c m", p=128))
                wr = singles.tile([128, NDC, HD2], bf16, tag=name)
                nc.vector.tensor_copy(wr[:], wf[:])
                return wr

            wq_sb = load_w("wq", wq)
            wk_sb = load_w("wk", wk)
            wv_sb = load_w("wv", wv)

            wo0_f = singles.tile([HD, D], f32, tag="wo0f")
            wo1_f = singles.tile([HD, D], f32, tag="wo1f")
            nc.sync.dma_start(wo0_f[:], wo0[:, :])
            nc.sync.dma_start(wo1_f[:], wo1[:, :])
            wo0_sb = singles.tile([HD, D], f32r, tag="wo0")
            wo1_sb = singles.tile([HD, D], f32r, tag="wo1")
            nc.vector.tensor_copy(wo0_sb[:], wo0_f[:])
            nc.vector.tensor_copy(wo1_sb[:], wo1_f[:])

            bq_sb = singles.tile([HD2, 1], f32, tag="bq")
            bk_sb = singles.tile([HD2, 1], f32, tag="bk")
            bvb_sb = singles.tile([128, HD2], f32, tag="bvb")
            mb_sb = singles.tile([KC, NKC], f32, tag="mb")
            nc.sync.dma_start(bq_sb[:], bq[:, :])
            nc.sync.dma_start(bk_sb[:], bk[:, :])
            nc.sync.dma_start(bvb_sb[:], bvb[:, :])
            nc.sync.dma_start(mb_sb[:], mb[:, :])

            # K^T [hd2, L] bf16 (one tile per L-block for fine-grained
            # deps) and V' [kpos, hd+1] f32r per (head, kpos-chunk)
            kt_t = [singles.tile([HD2, QB], bf16, tag=f"kt{i}", name=f"kt{i}")
                    for i in range(NQB)]
            v0_t = [singles.tile([128, HD + 1], f32r, tag=f"v0_{i}", name=f"v0_{i}")
                    for i in range(NKC)]
            v1_t = [singles.tile([128, HD + 1], f32r, tag=f"v1_{i}", name=f"v1_{i}")
                    for i in range(NKC)]
            for i in range(NKC):
                nc.vector.memset(v0_t[i][:, HD:HD + 1].bitcast(f32), 1.0)
                nc.vector.memset(v1_t[i][:, HD:HD + 1].bitcast(f32), 1.0)

            def load_x_block(dram, lb, tagp="x"):
                """DMA a [128, QB] f32 tile per D-chunk and cast to bf16."""
                tiles = []
                for dc in range(NDC):
                    xf = xload.tile([128, QB], f32, tag=tagp + "l", name="xf")
                    nc.sync.dma_start(
                        xf[:], dram[dc * 128:(dc + 1) * 128, lb * QB:(lb + 1) * QB])
                    xr = xcast.tile([128, QB], bf16, tag=tagp + "c", name="xr")
                    nc.vector.tensor_copy(xr[:], xf[:])
                    tiles.append(xr)
                return tiles

            def emit_kproj(lb, xts=None):
                if xts is None:
                    xts = load_x_block(xk, lb)
                ps_kt = ps_sp.tile([128, QB], f32, tag="pss", name="ps_kt")
                for dc in range(NDC):
                    nc.tensor.matmul(ps_kt[:], wk_sb[:, dc, :], xts[dc][:],
                                     start=(dc == 0), stop=(dc == NDC - 1))
                nc.vector.tensor_scalar_add(
                    kt_t[lb][:], in0=ps_kt[:], scalar1=bk_sb[:])

            def emit_vproj_lc(xts, lc):
                j = lc % 4
                ps_v = ps_sp.tile([128, HD2], f32, tag="pss", name="ps_v")
                for dc in range(NDC):
                    nc.tensor.matmul(
                        ps_v[:], xts[dc][:, j * 128:(j + 1) * 128],
                        wv_sb[:, dc, :],
                        start=(dc == 0), stop=(dc == NDC - 1))
                nc.vector.tensor_add(v0_t[lc][:, 0:HD], ps_v[:, 0:HD],
                                     bvb_sb[:, 0:HD])
                nc.vector.tensor_add(v1_t[lc][:, 0:HD], ps_v[:, HD:HD2],
                                     bvb_sb[:, HD:HD2])

            # startup: first q-block inputs + first K block; the rest of the
            # K/V projections are interleaved into q-block 0's chunk loop so
            # attention starts as soon as kt[0]/qt[0] land (~7us).
            q0_tiles = load_x_block(xq, 0, tagp='q0')
            emit_kproj(0)

            # ---------------- attention + output, pipelined per q-block ----
            pending = None  # (u0, u1, qb) awaiting normalize + Wo

            def emit_norm(u0, u1, qb):
                """Normalize u tiles -> xt0/xt1 (runs on DVE/DMA, lags PE)."""
                r0 = small.tile([HD + 1, QB], f32, tag="r0")
                r1 = small.tile([HD + 1, QB], f32, tag="r1")
                nc.vector.reciprocal(r0[HD:HD + 1, :], u0[HD:HD + 1, :])
                nc.vector.reciprocal(r1[HD:HD + 1, :], u1[HD:HD + 1, :])
                rb0 = small.tile([HD, QB], f32, tag="rb0")
                rb1 = small.tile([HD, QB], f32, tag="rb1")
                scr = dscr.tile([2, QB], f32, tag="scr")
                nc.sync.dma_start(scr[0:1, :], r0[HD:HD + 1, :])
                nc.sync.dma_start(scr[1:2, :], r1[HD:HD + 1, :])
                nc.sync.dma_start(rb0[:], scr[0:1, :].to_broadcast([HD, QB]))
                nc.sync.dma_start(rb1[:], scr[1:2, :].to_broadcast([HD, QB]))
                xt0 = xtp.tile([HD, QB], f32r, tag="xt0")
                xt1 = xtp.tile([HD, QB], f32r, tag="xt1")
                nc.vector.tensor_mul(xt0[:], u0[0:HD, :], rb0[:])
                nc.vector.tensor_mul(xt1[:], u1[0:HD, :], rb1[:])
                return (xt0, xt1, qb)

            def emit_wo(xt0, xt1, qb):
                for j in range(4):
                    qs = slice(j * 128, (j + 1) * 128)
                    ps_o = ps_sp.tile([128, D], f32, tag="pss")
                    nc.tensor.matmul(ps_o[:], xt0[:, qs], wo0_sb[:],
                                     start=True, stop=False)
                    nc.tensor.matmul(ps_o[:], xt1[:, qs], wo1_sb[:],
                                     start=False, stop=True)
                    o_t = op.tile([128, D], f32, tag="ot")
                    nc.vector.tensor_copy(o_t[:], ps_o[:])
                    nc.sync.dma_start(
                        out[qb * QB + j * 128: qb * QB + (j + 1) * 128, :], o_t[:])

            vx_tiles = None
            for qb in range(NQB):
                # Q projection for this q-block -> qt [hd2, QB] bf16
                xts = q0_tiles if qb == 0 else load_x_block(xq, qb)
                ps_q = ps_sp.tile([128, QB], f32, tag="pss")
                for dc in range(NDC):
                    nc.tensor.matmul(ps_q[:], wq_sb[:, dc, :], xts[dc][:],
                                     start=(dc == 0), stop=(dc == NDC - 1))
                qt = qtp.tile([HD2, QB], bf16, tag="qt")
                nc.vector.tensor_scalar_add(qt[:], in0=ps_q[:], scalar1=bq_sb[:])

                u0 = ps_up.tile([HD + 1, QB], f32, tag="u")
                u1 = ps_up.tile([HD + 1, QB], f32, tag="u")

                def emit_pv(pt, c):
                    nc.tensor.matmul(u0[:], v0_t[c][:], pt[:, 0:QB],
                                     start=(c == 0), stop=(c == NKC - 1))
                    nc.tensor.matmul(u1[:], v1_t[c][:], pt[:, QB:2 * QB],
                                     start=(c == 0), stop=(c == NKC - 1))

                pv_q = []
                norm_pending = None
                for c in range(NKC):
                    if qb == 0:
                        if c % 4 == 0:
                            if c < NKC - 4:
                                emit_kproj(c // 4 + 1)
                            vx_tiles = load_x_block(xv, c // 4)
                    kb, ko = c // 4, (c % 4) * KC
                    ks = slice(ko, ko + KC)
                    ps_s = ps_sp.tile([128, 2 * QB], f32, tag="pss")
                    nc.tensor.matmul(ps_s[:, 0:QB], kt_t[kb][0:HD, ks],
                                     qt[0:HD, :], start=True, stop=True)
                    nc.tensor.matmul(ps_s[:, QB:2 * QB], kt_t[kb][HD:HD2, ks],
                                     qt[HD:HD2, :], start=True, stop=True)
                    pt = ptp.tile([128, 2 * QB], f32r, tag="pt")
                    nc.scalar.activation(pt[:], ps_s[:], AF.Exp,
                                         bias=mb_sb[:, c:c + 1], scale=0.125)
                    if qb == 0:
                        emit_vproj_lc(vx_tiles, c)
                    pv_q.append((pt, c))
                    if len(pv_q) > 2:
                        emit_pv(*pv_q.pop(0))
                    if c == 4 and pending is not None:
                        norm_pending = emit_norm(*pending)
                        pending = None
                    if c == 12 and norm_pending is not None:
                        emit_wo(*norm_pending)
                        norm_pending = None
                for item in pv_q:
                    emit_pv(*item)
                pending = (u0, u1, qb)

            emit_wo(*emit_norm(*pending))

    nc.compile()
    return nc


def _get_nc():
    global _CACHED_NC
    if _CACHED_NC is None:
        _ensure_axon_hook()
        _CACHED_NC = _build_nc()
    return _CACHED_NC


def kernel(query, key, value, mask, Wq, bq, Wk, bk, Wv, bv, Wo, bo,
           _trace=False, _results_sink=None):
    from concourse.bass_utils import run_bass_kernel_spmd

    query = np.asarray(query, np.float32)
    key = np.asarray(key, np.float32)
    value = np.asarray(value, np.float32)
    mask = np.asarray(mask)
    Wq = np.asarray(Wq, np.float32)
    bq = np.asarray(bq, np.float32)
    Wk = np.asarray(Wk, np.float32)
    bk = np.asarray(bk, np.float32)
    Wv = np.asarray(Wv, np.float32)
    bv = np.asarray(bv, np.float32)
    Wo = np.asarray(Wo, np.float32)
    bo = np.asarray(bo, np.float32)

    nc = _get_nc()

    xqT = [np.ascontiguousarray(query[b].T) for b in range(B)]
    xkT = [np.ascontiguousarray(key[b].T) for b in range(B)]
    xvT = [np.ascontiguousarray(value[b].T) for b in range(B)]
    mbias = [
        np.ascontiguousarray(
            ((1 - mask[b].astype(np.float32)) * -1e30)
            .astype(np.float32).reshape(NKC, KC).T)
        for b in range(B)
    ]

    in_maps = []
    for core in range(NCORES):
        b = core // 4
        h0 = (core % 4) * HPC
        sl = slice(h0 * HD, (h0 + HPC) * HD)
        in_maps.append({
            "xq": xqT[b],
            "xk": xkT[b],
            "xv": xvT[b],
            "wq": np.ascontiguousarray(Wq[:, sl]),
            "wk": np.ascontiguousarray(Wk[:, sl]),
            "wv": np.ascontiguousarray(Wv[:, sl]),
            "wo0": np.ascontiguousarray(Wo[sl, :][0:HD]),
            "wo1": np.ascontiguousarray(Wo[sl, :][HD:HD2]),
            "bq": np.ascontiguousarray(bq[sl].reshape(HD2, 1)),
            "bk": np.ascontiguousarray(bk[sl].reshape(HD2, 1)),
            "bvb": np.ascontiguousarray(np.tile(bv[sl][None, :], (128, 1))),
            "mb": mbias[b],
        })

    res = run_bass_kernel_spmd(nc, in_maps, core_ids=list(range(NCORES)),
                               trace=_trace)
    if _results_sink is not None:
        _results_sink.append(res)

    final = np.empty((B, L, D), np.float32)
    for b in range(B):
        acc = res.results[4 * b]["out"].astype(np.float32).copy()
        for i in range(1, 4):
            acc += res.results[4 * b + i]["out"]
        final[b] = acc + bo[None, :]
    return final



# revision 12
# speedup vs baseline: 1.2412x; 1.2412x over previous
"""Multi-head attention (B=2, L=4096, D=512, H=8, HD=64) on 8 trn2 NeuronCores.

Sharding: data-parallel over batch (2) x tensor-parallel over head-pairs (4):
core c handles batch c//4, heads (c%4)*2 and (c%4)*2+1. Each core projects
Q/K/V for its two heads, runs flash-style attention (S^T orientation,
no-max-subtraction exp since logits are small), applies its rows of Wo, and
returns a partial [L, D] output. Host sums the 4 partials per batch, adds bo.

v2 design (Act-engine-bound pipeline):
- exp on the Act engine is the bottleneck (33.5M elems/core at 1 elem/lane/
  cycle @1.2GHz = 218us floor). The schedule keeps the Act exp stream
  gap-free: one [128, 1024] Exp per kpos-chunk, double-buffered scores psum.
- PV re-oriented: stationary = exp(S^T) chunk [128k x 128q] bf16 (gets FWL),
  moving = V' [128, 65] bf16 (64 hd cols + ones column for the softmax
  denominator), accumulating x[q, hd] + denom in PSUM over the 32 k-chunks.
  Streams N=65 cycles/matmul vs 512 in the v1 orientation (2x less PE).
- No input casts: projections stream x tiles as f32r moving (full PE rate at
  N=512); V is projected transposed ([hd2, kpos]) then PE-transposed into V'.
- Normalization is per-partition (q on partitions after PV): reciprocal of
  the ones-column then tensor_scalar mult; no DRAM broadcast roundtrip.
- Software pipeline with a one-window phase shift: PV/normalize/Wo for query
  block qb run during window qb+1/qb+2, which also spreads the K/V load DMA
  (the prologue would otherwise need ~470 GB/s).
"""

import sys
import types

import numpy as np

B, L, D = 2, 4096, 512
H, HD = 8, 64
NCORES = 8
HPC = 2          # heads per core
HD2 = HPC * HD   # 128
QB = 512         # query block
NQB = L // QB    # 8
KC = 128         # key-position chunk (partition dim of S^T tiles)
NKC = L // KC    # 32
NDC = D // 128   # contraction chunks for projections

# chunks whose exp runs on DVE (Schraudolph bf16 bit-trick) instead of Act.
# Empty in phase 1; filled in by tuning.
DVE_EXP_CS = ()
# Schraudolph constants for bf16: bits = round(s_raw * SCH_C1 + SCH_C2)
SCH_C1 = 128.0 * 0.125 * 1.4426950408889634
SCH_C2 = 16250.5

_CACHED_NC = None


def _ensure_axon_hook():
    """Register the NTFF profile hook boot() couldn't (stub antenv lacks
    axon_hooks). Harmless when tracing is never requested."""
    try:
        from antenv.axon_hooks import get_axon_ntff_profile_hook  # noqa: F401
        return
    except ImportError:
        pass
    hook = None
    try:
        from trn_agent_boot.trn_boot import _ntff_profile_via_ctypes
        hook = _ntff_profile_via_ctypes("/opt/axon/libaxon_pjrt.so")
    except Exception:
        pass
    mod = types.ModuleType("antenv.axon_hooks")
    mod.get_axon_ntff_profile_hook = lambda: hook
    mod.set_axon_ntff_profile_hook = lambda h: None
    sys.modules["antenv.axon_hooks"] = mod


def _build_nc():
    from concourse import bacc
    import concourse.mybir as mybir
    import concourse.tile as tile

    f32 = mybir.dt.float32
    f32r = mybir.dt.float32r
    bf16 = mybir.dt.bfloat16
    i16 = mybir.dt.int16
    AF = mybir.ActivationFunctionType

    nc = bacc.Bacc("TRN2", target_bir_lowering=False, debug=False,
                   num_devices=NCORES)

    xq = nc.dram_tensor("xq", [D, L], f32r, kind="ExternalInput")
    xk = nc.dram_tensor("xk", [D, L], f32r, kind="ExternalInput")
    xv = nc.dram_tensor("xv", [D, L], f32r, kind="ExternalInput")
    wq = nc.dram_tensor("wq", [D, HD2], f32r, kind="ExternalInput")
    wk = nc.dram_tensor("wk", [D, HD2], f32r, kind="ExternalInput")
    wv = nc.dram_tensor("wv", [D, HD2], f32r, kind="ExternalInput")
    wo = nc.dram_tensor("wo", [HD2, D], f32, kind="ExternalInput")
    bq = nc.dram_tensor("bq", [HD2, 1], f32, kind="ExternalInput")
    bk = nc.dram_tensor("bk", [HD2, 1], f32, kind="ExternalInput")
    bv = nc.dram_tensor("bv", [HD2, 1], f32, kind="ExternalInput")
    mb = nc.dram_tensor("mb", [KC, NKC], f32, kind="ExternalInput")
    ident = nc.dram_tensor("ident", [128, 128], bf16, kind="ExternalInput")
    out = nc.dram_tensor("out", [L, D], f32, kind="ExternalOutput")

    with tile.TileContext(nc) as tc:
        with (
            tc.tile_pool(name="singles", bufs=1) as singles,
            tc.tile_pool(name="xload", bufs=5) as xload,
            tc.tile_pool(name="qtp", bufs=2) as qtp,
            tc.tile_pool(name="vtp", bufs=2) as vtp,
            tc.tile_pool(name="ptp", bufs=NKC + 2) as ptp,
            tc.tile_pool(name="xsp", bufs=2) as xsp,
            tc.tile_pool(name="xtp", bufs=2) as xtp,
            tc.tile_pool(name="dnp", bufs=2) as dnp,
            tc.tile_pool(name="otp", bufs=3) as otp,
            tc.tile_pool(name="ps_s", bufs=2, space="PSUM") as ps_sp,
            tc.tile_pool(name="ps_u", bufs=2, space="PSUM") as ps_up,
            tc.tile_pool(name="ps_w", bufs=2, space="PSUM") as ps_wp,
        ):
            # ---------------- weights / constants (f32 bits reused as f32r) --
            def load_w(name, dram):
                wf = singles.tile([128, NDC, HD2], f32r, tag=name)
                nc.sync.dma_start(wf[:], dram.rearrange("(c p) m -> p c m", p=128))
                return wf

            wq_sb = load_w("wq", wq)
            wk_sb = load_w("wk", wk)
            wv_sb = load_w("wv", wv)

            wo_f = singles.tile([HD, HPC, D], f32, tag="wof")
            for h in range(HPC):
                nc.sync.dma_start(wo_f[:, h, :], wo[h * HD:(h + 1) * HD, :])
            wo_sb = singles.tile([HD, HPC, D], bf16, tag="wo")
            nc.vector.tensor_copy(wo_sb[:], wo_f[:])

            bq_sb = singles.tile([HD2, 1], f32, tag="bq")
            bk_sb = singles.tile([HD2, 1], f32, tag="bk")
            bv_sb = singles.tile([HD2, 1], f32, tag="bv")
            mb_sb = singles.tile([KC, NKC], f32, tag="mb")
            id_sb = singles.tile([128, 128], bf16, tag="ident")
            nc.sync.dma_start(bq_sb[:], bq[:, :])
            nc.sync.dma_start(bk_sb[:], bk[:, :])
            nc.sync.dma_start(bv_sb[:], bv[:, :])
            nc.sync.dma_start(mb_sb[:], mb[:, :])
            nc.sync.dma_start(id_sb[:], ident[:, :])

            # K^T [hd2, kpos] bf16 per 512-block; V' [kpos, (h, hd+1)] bf16
            # per kpos-chunk with a ones column for the softmax denominator.
            kt_t = [singles.tile([HD2, QB], bf16, tag=f"kt{i}", name=f"kt{i}")
                    for i in range(NQB)]
            v_t = [singles.tile([128, HPC, HD + 1], bf16, tag=f"v{i}",
                                name=f"v{i}")
                   for i in range(NKC)]
            for i in range(NKC):
                nc.vector.memset(v_t[i][:, :, HD:HD + 1], 1.0)

            def load_x_block(dram, lb, tagp="x"):
                xf = xload.tile([128, NDC, QB], f32r, tag="xl", name=tagp)
                for dc in range(NDC):
                    nc.sync.dma_start(
                        xf[:, dc, :],
                        dram[dc * 128:(dc + 1) * 128, lb * QB:(lb + 1) * QB])
                return xf

            def emit_proj(w_sb, xf, ps):
                for dc in range(NDC):
                    nc.tensor.matmul(ps[:], w_sb[:, dc, :], xf[:, dc, :],
                                     start=(dc == 0), stop=(dc == NDC - 1))

            def emit_kproj(lb):
                xf = load_x_block(xk, lb, tagp="xk")
                ps = ps_wp.tile([128, QB], f32, tag="psw", name="ps_k")
                emit_proj(wk_sb, xf, ps)
                nc.vector.tensor_scalar_add(kt_t[lb][:], in0=ps[:],
                                            scalar1=bk_sb[:])

            def emit_vproj(lb):
                """Project V transposed ([hd2, kpos]) then PE-transpose into
                the 4 per-chunk V' tiles."""
                xf = load_x_block(xv, lb, tagp="xv")
                ps = ps_wp.tile([128, QB], f32, tag="psw", name="ps_v")
                emit_proj(wv_sb, xf, ps)
                vt = vtp.tile([128, QB], bf16, tag="vt")
                nc.vector.tensor_scalar_add(vt[:], in0=ps[:], scalar1=bv_sb[:])
                for j in range(4):
                    pst = ps_wp.tile([128, 128], bf16, tag="psw", name="ps_vt")
                    nc.tensor.transpose(pst[:], vt[:, j * 128:(j + 1) * 128],
                                        id_sb[:])
                    c = lb * 4 + j
                    for h in range(HPC):
                        nc.vector.tensor_copy(
                            v_t[c][:, h, 0:HD],
                            pst[:, h * HD:(h + 1) * HD])

            def emit_qproj(qb):
                xf = load_x_block(xq, qb, tagp="xq")
                ps = ps_wp.tile([128, QB], f32, tag="psw", name="ps_q")
                emit_proj(wq_sb, xf, ps)
                qt = qtp.tile([HD2, QB], bf16, tag="qt")
                nc.vector.tensor_scalar_add(qt[:], in0=ps[:], scalar1=bq_sb[:])
                return qt

            def emit_scores(qt, c):
                ps = ps_sp.tile([128, HPC, QB], f32, tag="pss")
                kb, ko = c // 4, (c % 4) * KC
                for h in range(HPC):
                    nc.tensor.matmul(
                        ps[:, h, :],
                        kt_t[kb][h * HD:(h + 1) * HD, ko:ko + KC],
                        qt[h * HD:(h + 1) * HD, :], start=True, stop=True)
                return ps

            def emit_exp(ps, c):
                pt = ptp.tile([128, HPC, QB], bf16, tag="pt")
                if c in DVE_EXP_CS:
                    nc.vector.tensor_scalar(
                        out=pt[:].bitcast(i16), in0=ps[:],
                        scalar1=SCH_C1, scalar2=SCH_C2,
                        op0=mybir.AluOpType.mult, op1=mybir.AluOpType.add)
                else:
                    nc.scalar.activation(pt[:], ps[:], AF.Exp,
                                         bias=mb_sb[:, c:c + 1], scale=0.125)
                return pt

            def emit_pv(pt, c, u):
                last = c == NKC - 1
                for h in range(HPC):
                    for qc in range(4):
                        # start=True zeroes the whole 2KB PSUM bank (the u[h]
                        # tile), so only the first sub-region write may carry
                        # it; the siblings' first writes land on bank bytes
                        # still marked pending-zero and overwrite correctly.
                        nc.tensor.matmul(
                            u[h][:, qc, :],
                            pt[:, h, qc * 128:(qc + 1) * 128],
                            v_t[c][:, h, :],
                            start=(c == 0 and qc == 0), stop=last)

            def emit_norm(u):
                """u: [u0, u1] PSUM [128, 4, HD+1] -> xs [128, 2, 4, HD] bf16
                normalized by the accumulated ones column."""
                dn = dnp.tile([128, 2, 4], f32, tag="dn")
                for h in range(HPC):
                    nc.vector.tensor_copy(dn[:, h, :], u[h][:, :, HD:HD + 1])
                rc = dnp.tile([128, 2, 4], f32, tag="rc")
                nc.vector.reciprocal(rc[:], dn[:])
                xs = xsp.tile([128, HPC, 4, HD], bf16, tag="xs")
                for h in range(HPC):
                    for qc in range(4):
                        nc.vector.tensor_scalar_mul(
                            xs[:, h, qc, :], in0=u[h][:, qc, 0:HD],
                            scalar1=rc[:, h, qc:qc + 1])
                return xs

            def emit_xpose(xs):
                pst = ps_wp.tile([64, HPC, 4, 128], bf16, tag="psw",
                                 name="ps_xt")
                for h in range(HPC):
                    for qc in range(4):
                        nc.tensor.transpose(pst[:, h, qc, :], xs[:, h, qc, :],
                                            id_sb[:])
                xt = xtp.tile([64, HPC, 4, 128], bf16, tag="xt")
                nc.vector.tensor_copy(xt[:], pst[:])
                return xt

            def emit_wo_j(xt, qb, j):
                ps = ps_wp.tile([128, D], f32, tag="psw", name="ps_o")
                nc.tensor.matmul(ps[:], xt[:, 0, j, :], wo_sb[:, 0, :],
                                 start=True, stop=False)
                nc.tensor.matmul(ps[:], xt[:, 1, j, :], wo_sb[:, 1, :],
                                 start=False, stop=True)
                o_t = otp.tile([128, D], f32, tag="ot")
                nc.vector.tensor_copy(o_t[:], ps[:])
                nc.sync.dma_start(
                    out[qb * QB + j * 128: qb * QB + (j + 1) * 128, :], o_t[:])

            # ---------------- pipelined schedule ----------------
            # window w (w = 0..NQB-1): scores+exp for qb=w, PV for qb=w-1,
            # norm/xpose for qb=w-2 at c==0/1, Wo for qb=w-2 at c==2,4,6,8,
            # qproj for qb=w+1 at c==26. K/V proj interleaved into window 0.
            # windows NQB, NQB+1 drain the tail.
            qt_cur = emit_qproj(0)
            emit_kproj(0)

            qt_next = None
            u_cur = None        # PV accumulators for qb = w-1
            xs_pend = None      # normalized x for qb = w-2
            xt_pend = None      # (xt, qb) pending Wo
            pt_hist = {}        # (qb, c) -> pt tile

            for w in range(NQB + 1):
                exp_w = w < NQB
                for c in range(NKC):
                    # prologue interleave (window 0): K/V projections JIT
                    if w == 0:
                        if c % 4 == 1 and c // 4 + 1 < NQB:
                            emit_kproj(c // 4 + 1)
                        if c % 4 == 3:
                            emit_vproj(c // 4)
                    # norm/xpose/Wo for qb = w-2
                    if c == 0 and xs_pend is not None:
                        xt_pend = (emit_xpose(xs_pend[0]), xs_pend[1])
                        xs_pend = None
                    if c in (2, 4, 6, 8) and xt_pend is not None:
                        emit_wo_j(xt_pend[0], xt_pend[1], (c - 2) // 2)
                        if c == 8:
                            xt_pend = None
                    if c == 26 and w + 1 < NQB:
                        qt_next = emit_qproj(w + 1)

                    # PV for qb = w-1 (phase-shifted one window)
                    if w >= 1:
                        emit_pv(pt_hist.pop((w - 1, c)), c, u_cur)
                        if c == NKC - 1:
                            xs_pend = (emit_norm(u_cur), w - 1)
                            u_cur = None

                    # scores + exp for qb = w
                    if exp_w:
                        if c == 0:
                            u_next = [
                                ps_up.tile([128, 4, HD + 1], f32, tag="u",
                                           name=f"u{h}") for h in range(HPC)]
                        ps = emit_scores(qt_cur, c)
                        pt_hist[(w, c)] = emit_exp(ps, c)
                if exp_w:
                    u_cur = u_next
                    qt_cur = qt_next
            # tail: norm/xpose/Wo for the last qb
            xt_last = emit_xpose(xs_pend[0])
            for j in range(4):
                emit_wo_j(xt_last, xs_pend[1], j)

    nc.compile()
    return nc


def _get_nc():
    global _CACHED_NC
    if _CACHED_NC is None:
        _ensure_axon_hook()
        _CACHED_NC = _build_nc()
    return _CACHED_NC


def kernel(query, key, value, mask, Wq, bq, Wk, bk, Wv, bv, Wo, bo,
           _trace=False, _results_sink=None):
    import ml_dtypes
    from concourse.bass_utils import run_bass_kernel_spmd

    query = np.asarray(query, np.float32)
    key = np.asarray(key, np.float32)
    value = np.asarray(value, np.float32)
    mask = np.asarray(mask)
    Wq = np.asarray(Wq, np.float32)
    bq = np.asarray(bq, np.float32)
    Wk = np.asarray(Wk, np.float32)
    bk = np.asarray(bk, np.float32)
    Wv = np.asarray(Wv, np.float32)
    bv = np.asarray(bv, np.float32)
    Wo = np.asarray(Wo, np.float32)
    bo = np.asarray(bo, np.float32)

    nc = _get_nc()

    xqT = [np.ascontiguousarray(query[b].T) for b in range(B)]
    xkT = [np.ascontiguousarray(key[b].T) for b in range(B)]
    xvT = [np.ascontiguousarray(value[b].T) for b in range(B)]
    mbias = [
        np.ascontiguousarray(
            ((1 - mask[b].astype(np.float32)) * -1e30)
            .astype(np.float32).reshape(NKC, KC).T)
        for b in range(B)
    ]
    ident = np.eye(128, dtype=ml_dtypes.bfloat16)

    in_maps = []
    for core in range(NCORES):
        b = core // 4
        h0 = (core % 4) * HPC
        sl = slice(h0 * HD, (h0 + HPC) * HD)
        in_maps.append({
            "xq": xqT[b],
            "xk": xkT[b],
            "xv": xvT[b],
            "wq": np.ascontiguousarray(Wq[:, sl]),
            "wk": np.ascontiguousarray(Wk[:, sl]),
            "wv": np.ascontiguousarray(Wv[:, sl]),
            "wo": np.ascontiguousarray(Wo[sl, :]),
            "bq": np.ascontiguousarray(bq[sl].reshape(HD2, 1)),
            "bk": np.ascontiguousarray(bk[sl].reshape(HD2, 1)),
            "bv": np.ascontiguousarray(bv[sl].reshape(HD2, 1)),
            "mb": mbias[b],
            "ident": ident,
        })

    res = run_bass_kernel_spmd(nc, in_maps, core_ids=list(range(NCORES)),
                               trace=_trace)
    if _results_sink is not None:
        _results_sink.append(res)

    final = np.empty((B, L, D), np.float32)
    for b in range(B):
        acc = res.results[4 * b]["out"].astype(np.float32).copy()
        for i in range(1, 4):
            acc += res.results[4 * b + i]["out"]
        final[b] = acc + bo[None, :]
    return final
